# revision 1
# baseline (speedup 1.0000x reference)
"""Trainium2 Bass kernel for AttentiveSSMNoProjCyc (sparse_attention).

Sharding: 8 cores = 2 batches x 4 head-groups (4 heads / 256 channels each).
Per core, everything lives in transposed [channel, time] layout:
  - SSM scans via tensor_tensor_scan (DVE), resets folded in as A=0 cols
  - RoPE via stream_shuffle (partition pair-swap) + general 2x2 coeffs
  - local band (k in {t-1,t}) scores computed elementwise + ones-matmul
    partition reduction; 33 boundary keys gathered + small matmuls
  - output projection produces a partial over full D; host sums the 4
    partials per batch and transposes back.
Host-side work is limited to slicing / transposition / constant tables.
"""
import numpy as np

import concourse.bass as bass
import concourse.mybir as mybir
from concourse.bass_utils import run_bass_kernel_spmd
from concourse.tile import TileContext
import concourse.tile as _tile_mod
from concourse.vector_clock import ScopedClock as _ScopedClock


def _split_drain_and_barrier(self, tick_clock, wait_clock):
    """Tail drain, with its sem waits spread over chained SP nops.

    The stock tail drain can accumulate one wait per logical processor;
    walrus's TPB_CTRL lowering only accepts a couple of sync waits per
    instruction, so redistribute them one-per-nop (same engine, program
    order => semantics preserved).
    """
    probe = self.nc.sync.nop()
    wait_clock.add_sem_waits(
        probe.ins, _ScopedClock({None: tick_clock.global_clock})
    )
    si = probe.ins.sync_info
    waits = list(si.on_wait) if si is not None else []
    upds = list(si.on_update) if si is not None else []
    MAXW = 1
    if len(waits) > MAXW:
        probe.ins.sync_info = mybir.SyncInfo(on_wait=waits[:MAXW],
                                             on_update=upds)
        for i in range(MAXW, len(waits), MAXW):
            extra = self.nc.sync.nop()
            extra.ins.sync_info = mybir.SyncInfo(
                on_wait=waits[i:i + MAXW], on_update=[])
    self.nc.sync.drain()

    self.nc.all_engine_barrier()
    assert self.sems is not None
    popped = self.nc._tile_sem_poison_stack.pop()
    assert popped is self._sem_poison
    self.nc.clear_and_free_semaphores(list(self.sems.allocated().values()))
    self.nc.all_engine_barrier()


_tile_mod.TileContext._drain_and_barrier = _split_drain_and_barrier


def _cap_sync_waits(nc, cap=1):
    """Walrus only accepts `cap` sync waits per instruction; hoist the
    excess onto same-engine carrier NOPs inserted immediately before the
    overloaded instruction (same engine + program order => semantics
    preserved, the nop just stalls in place of the instruction)."""
    nid = [0]

    def mknop(engine, waits):
        nid[0] += 1
        nop = mybir.InstNoOp(name=f"I-capw-{nid[0]}", ins=[], outs=[])
        nop.engine = engine
        nop.sync_info = mybir.SyncInfo(on_wait=list(waits), on_update=[])
        return nop

    for bb in nc.m.functions[0].blocks:
        il = bb.instructions
        i = 0
        while i < len(il):
            ins = il[i]
            si = ins.sync_info
            nw = len(si.on_wait) if si is not None else 0
            if nw > cap:
                waits = list(si.on_wait)
                ins.sync_info = mybir.SyncInfo(on_wait=waits[:cap],
                                               on_update=list(si.on_update))
                rest = waits[cap:]
                pos = i
                for j in range(0, len(rest), cap):
                    il.insert(pos, mknop(ins.engine, rest[j:j + cap]))
                    pos += 1
                    i += 1
            i += 1


B, S, D, H, HD = 2, 2048, 1024, 16, 64
NG = 4            # head-groups per batch
CH = 256          # channels per core (4 heads)
NB = 48           # padded boundary columns (33 real)
NBD = 112         # blockdiag boundary cols: head0 -> 0:48, head1 -> 64:112
NCHUNK = 4
CS = S // NCHUNK  # 512
F32 = mybir.dt.float32
F32R = mybir.dt.float32r
AL = mybir.AluOpType
AF = mybir.ActivationFunctionType
NEG = -1e30


def _boundaries():
    K_, LAYER_, NLAYERS_, MAXLEN_ = 64, 4, 16, 16384
    off = min(K_ - 1, LAYER_ * (K_ // NLAYERS_))
    bl = [b - off for b in range(K_ - 1, MAXLEN_, K_)]
    if bl[-1] != MAXLEN_ - 1:
        bl.append(MAXLEN_ - 1)
    if bl[0] != 0:
        bl.insert(0, 0)
    b = np.asarray(bl)
    b = b[b < S].copy()
    b[-1] = S - 1
    return b


BND = _boundaries()
NBR = len(BND)  # 33


def build_program():
    nc = bass.Bass()
    dp = nc.declare_dram_parameter
    xt = dp("xt", [D, S], F32, isOutput=False)
    wqt = dp("wqt", [D, CH], F32, isOutput=False)
    wot = dp("wot", [CH, D], F32, isOutput=False)
    cbc = dp("cbc", [128, S], F32, isOutput=False)
    sbc = dp("sbc", [128, S], F32, isOutput=False)
    nrs = dp("nrs", [128, S], F32, isOutput=False)
    apar = dp("apar", [128, 4], F32, isOutput=False)
    oblk = dp("oblk", [128, 8], F32, isOutput=False)    # cols 0-3 dt0, 4-7 dt1
    obv = dp("obv", [NBD, 8], F32, isOutput=False)      # boundary denom ones
    ind = dp("ind", [128, 256], F32, isOutput=False)      # cols 0-127 dt0, rest dt1
    maskb = dp("maskb", [NBD, S], F32, isOutput=False)
    ident = dp("ident", [128, 128], F32, isOutput=False)
    outp = dp("outp", [D, S], F32, isOutput=True)

    SHUF_XOR1 = [i ^ 1 for i in range(32)]

    with TileContext(nc) as tc:
        with (
            tc.tile_pool(name="persist", bufs=1) as pp,
            tc.tile_pool(name="shared", bufs=3) as sh,     # cbc/sbc->embd/maskb
            tc.tile_pool(name="scratch", bufs=3) as sc,    # A/h/kpre/rope 8K
            tc.tile_pool(name="xop", bufs=2) as xop,
            tc.tile_pool(name="prods", bufs=2) as prp,
            tc.tile_pool(name="late", bufs=3) as lt,       # xq/qr/attn reuse
            tc.tile_pool(name="xs", bufs=2) as xs,        # Qproj x stream
            tc.tile_pool(name="mm", bufs=5) as mpool,      # combine m1..m4
            tc.tile_pool(name="small", bufs=2) as ck,
            tc.tile_pool(name="outsb", bufs=2) as osb,
            tc.tile_pool(name="psA", bufs=3, space="PSUM") as psA,
            tc.tile_pool(name="psB", bufs=4, space="PSUM") as psB,
        ):
            # ============ constants ============
            cbc_t = sh.tile([128, S], F32, tag="cs_embd")
            sbc_t = sh.tile([128, S], F32, tag="cs_embd")
            nrs_t = sh.tile([128, S], F32, tag="cs_embd", name="nrs_t")
            nc.sync.dma_start(out=cbc_t, in_=cbc[:, :])
            nc.sync.dma_start(out=sbc_t, in_=sbc[:, :])
            nc.sync.dma_start(out=nrs_t, in_=nrs[:, :])
            apar_t = pp.tile([128, 4], F32, tag="apar")
            oblk_t = pp.tile([128, 8], F32, tag="oblk")
            obv_t = pp.tile([NBD, 8], F32, tag="obv")
            ind_t = pp.tile([128, 256], F32, tag="ind")
            ident_t = pp.tile([128, 128], F32, tag="ident")
            nc.sync.dma_start(out=apar_t, in_=apar[:, :])
            nc.sync.dma_start(out=oblk_t, in_=oblk[:, :])
            nc.sync.dma_start(out=obv_t, in_=obv[:, :])
            nc.sync.dma_start(out=ind_t, in_=ind[:, :])
            nc.sync.dma_start(out=ident_t, in_=ident[:, :])

            asig = pp.tile([128, 4], F32, tag="asig")
            nc.scalar.activation(asig, apar_t, AF.Sigmoid)
            oma = pp.tile([128, 4], F32, tag="oma")  # 1 - sigmoid(a)
            nc.vector.tensor_scalar(out=oma, in0=asig, scalar1=-1.0,
                                    scalar2=1.0, op0=AL.mult, op1=AL.add)

            # ============ weights ============
            wqt_t = [pp.tile([128, CH], F32R, tag=f"wqt{k}", name=f"wqt_t{k}") for k in range(8)]
            for k in range(8):
                nc.gpsimd.dma_start(out=wqt_t[k], in_=wqt[k * 128:(k + 1) * 128, :])
            wot_t = [pp.tile([128, D], F32R, tag=f"wot{k}", name=f"wot_t{k}") for k in range(2)]
            for k in range(2):
                nc.gpsimd.dma_start(out=wot_t[k], in_=wot[k * 128:(k + 1) * 128, :])

            # ============ SSM + rope-k (interleaved for SBUF reuse) ============
            x_own = [xop.tile([128, S], F32, tag="xo_prod", name=f"x_own{i}") for i in range(2)]
            for dt in range(2):
                nc.sync.dma_start(out=x_own[dt], in_=xt[dt * 128:(dt + 1) * 128, :])
            v_t = [pp.tile([128, S], F32, tag=f"v{dt}", name=f"v_t{dt}") for dt in range(2)]
            kr = [pp.tile([128, S], F32, tag=f"kr{dt}", name=f"kr{dt}") for dt in range(2)]
            for dt in range(2):
                # k scan -> kpre -> rope -> kr
                acol = asig[:, dt:dt + 1]
                ocol = oma[:, dt:dt + 1]
                A_t = sc.tile([128, S], F32, tag="sc8k")
                nc.scalar.activation(A_t, nrs_t, AF.Copy, scale=acol)
                h_t = sc.tile([128, S], F32, tag="sc8k")
                nc.vector.tensor_tensor_scan(out=h_t, data0=A_t,
                                             data1=x_own[dt], initial=0.0,
                                             op0=AL.mult, op1=AL.add)
                kpre = sc.tile([128, S], F32, tag="sc8k")
                nc.vector.scalar_tensor_tensor(out=kpre, in0=h_t, scalar=ocol,
                                               in1=x_own[dt], op0=AL.mult,
                                               op1=AL.add)
                zs = sc.tile([128, S], F32, tag="sc8k")
                nc.vector.stream_shuffle(zs, kpre, SHUF_XOR1)
                t1 = sc.tile([128, S], F32, tag="sc8k")
                nc.vector.tensor_mul(out=t1, in0=kpre, in1=cbc_t)
                t2 = sc.tile([128, S], F32, tag="sc8k")
                nc.vector.tensor_mul(out=t2, in0=zs, in1=sbc_t)
                nc.vector.tensor_add(out=kr[dt], in0=t1, in1=t2)
                # v scan
                acol = asig[:, 2 + dt:3 + dt]
                ocol = oma[:, 2 + dt:3 + dt]
                A_t = sc.tile([128, S], F32, tag="sc8k")
                nc.scalar.activation(A_t, nrs_t, AF.Copy, scale=acol)
                h_t = sc.tile([128, S], F32, tag="sc8k")
                nc.vector.tensor_tensor_scan(out=h_t, data0=A_t,
                                             data1=x_own[dt], initial=0.0,
                                             op0=AL.mult, op1=AL.add)
                nc.vector.scalar_tensor_tensor(out=v_t[dt], in0=h_t,
                                               scalar=ocol, in1=x_own[dt],
                                               op0=AL.mult, op1=AL.add)

            # ============ Q projection (streamed by t-chunk) ============
            xq = [lt.tile([128, S], F32, tag="xq_attn", name=f"xq{i}") for i in range(2)]
            for c in range(NCHUNK):
                accs = [psA.tile([128, CS], F32, tag="mmacc", name=f"qacc{c}_{m}")
                        for m in range(2)]
                for k in range(8):
                    xtr = xs.tile([128, CS], F32R, tag="xtr", name=f"xtr_c{c}_{k}", bufs=4)
                    nc.gpsimd.dma_start(out=xtr,
                                        in_=xt[k * 128:(k + 1) * 128,
                                               c * CS:(c + 1) * CS])
                    for m in range(2):
                        nc.tensor.matmul(accs[m],
                                         wqt_t[k][:, m * 128:(m + 1) * 128],
                                         xtr, start=(k == 0), stop=(k == 7))
                for m in range(2):
                    nc.scalar.activation(xq[m][:, c * CS:(c + 1) * CS], accs[m],
                                         AF.Copy)

            # ============ RoPE on q ============
            qr = [lt.tile([128, S], F32, tag="xq_attn", name=f"qr{dt}") for dt in range(2)]
            for dt in range(2):
                zs = sc.tile([128, S], F32, tag="sc8k")
                nc.vector.stream_shuffle(zs, xq[dt], SHUF_XOR1)
                t1 = sc.tile([128, S], F32, tag="sc8k")
                nc.vector.tensor_mul(out=t1, in0=xq[dt], in1=cbc_t)
                t2 = sc.tile([128, S], F32, tag="sc8k")
                nc.vector.tensor_mul(out=t2, in0=zs, in1=sbc_t)
                nc.vector.tensor_add(out=qr[dt], in0=t1, in1=t2)

            # ============ band scores ============
            rows4 = pp.tile([4, 3 * S], F32, tag="rows4")
            # cols [0:S] = e1 (-> p1 in place), [S:2S] = e0 (-> p0),
            # [2S:3S] = den (-> rd in place)
            e1r = rows4[:, 0:S]
            e0r = rows4[:, S:2 * S]
            denr = rows4[:, 2 * S:3 * S]
            for dt in range(2):
                prod1 = prp.tile([128, S], F32, tag="prod")
                nc.vector.tensor_mul(out=prod1, in0=qr[dt], in1=kr[dt])
                prod0 = prp.tile([128, S], F32, tag="prod")
                nc.vector.memset(prod0[:, 0:1], 0.0)
                nc.vector.tensor_mul(out=prod0[:, 1:S], in0=qr[dt][:, 1:S],
                                     in1=kr[dt][:, 0:S - 1])
                for c in range(NCHUNK):
                    chs = slice(c * CS, (c + 1) * CS)
                    for pi, pr in ((0, prod1), (1, prod0)):
                        sp = psB.tile([128, CS], F32, tag="psb")
                        nc.tensor.matmul(sp[0:4, :], oblk_t[:, 4 * dt:4 * dt + 4],
                                         pr[:, chs], start=True, stop=True)
                        if c == 0 and pi == 1:
                            nc.vector.memset(sp[0:4, 0:1], NEG)
                        dstr = e1r if pi == 0 else e0r
                        if dt == 0:
                            nc.scalar.activation(dstr[:, chs], sp[0:4, :],
                                                 AF.Exp, scale=0.125)
                        else:
                            eb = ck.tile([128, CS], F32, tag="ebch")
                            nc.scalar.activation(eb[0:4, :], sp[0:4, :],
                                                 AF.Exp, scale=0.125)
                            nc.vector.tensor_mul(out=dstr[:, chs],
                                                 in0=dstr[:, chs],
                                                 in1=eb[0:4, :])
            # e(dt0) * e(dt1): heads whose channels live in the other d-tile
            # see exp(0)=1 there, so the product equals exp(s_h).

            # ============ boundary keys ============
            embd = [sh.tile([128, S], F32, tag="cs_embd", name=f"embd{i}") for i in range(2)]
            maskb_t = sh.tile([128, S], F32, tag="cs_embd")
            nc.sync.dma_start(out=maskb_t[0:NBD, :], in_=maskb[:, :])
            vbT = [pp.tile([128, 64], F32, tag=f"vbT{dt}", name=f"vbT{dt}") for dt in range(2)]
            for dt in range(2):
                kb = ck.tile([128, NB], F32, tag="kb")
                vb = ck.tile([128, NB], F32, tag="vb")
                for src_t, dst_t in ((kr[dt], kb), (v_t[dt], vb)):
                    nc.vector.tensor_copy(out=dst_t[:, 0:1], in_=src_t[:, 0:1])
                    nc.vector.tensor_copy(
                        out=dst_t[:, 1:32],
                        in_=src_t.rearrange("p (a b) -> p a b", b=64)[:, 0:31, 47])
                    nc.vector.tensor_copy(out=dst_t[:, 32:33],
                                          in_=src_t[:, S - 1:S])
                    nc.vector.memset(dst_t[:, 33:NB], 0.0)
                kbd = ck.tile([128, NBD], F32, tag="kbd")
                nc.vector.memset(kbd, 0.0)
                nc.vector.tensor_copy(out=kbd[0:64, 0:48], in_=kb[0:64, :])
                nc.vector.tensor_copy(out=kbd[64:128, 64:112], in_=kb[64:128, :])
                for hh in range(2):
                    tp = psB.tile([128, CS], F32, tag="psb")
                    nc.tensor.transpose(tp[0:48, 0:64],
                                        vb[hh * 64:(hh + 1) * 64, 0:48],
                                        ident_t[hh * 64:(hh + 1) * 64,
                                                hh * 64:(hh + 1) * 64],
                                        tile_position=(hh * 64, 0))
                    nc.scalar.activation(vbT[dt][hh * 64:hh * 64 + 48, :],
                                         tp[0:48, 0:64], AF.Copy)
                for c in range(NCHUNK):
                    chs = slice(c * CS, (c + 1) * CS)
                    sb = psB.tile([128, CS], F32, tag="psb")
                    nc.tensor.matmul(sb[0:NBD, :], kbd, qr[dt][:, chs],
                                     start=True, stop=True)
                    nc.scalar.activation(embd[dt][0:NBD, chs], sb[0:NBD, :],
                                         AF.Exp, scale=0.125)
                    nc.vector.tensor_mul(out=embd[dt][0:NBD, chs],
                                         in0=embd[dt][0:NBD, chs],
                                         in1=maskb_t[0:NBD, chs])

            # ============ denominator rows ============
            den = denr
            rd = denr
            p1 = e1r
            p0 = e0r
            nc.vector.tensor_add(out=den, in0=e1r, in1=e0r)
            for c in range(NCHUNK):
                chs = slice(c * CS, (c + 1) * CS)
                db = psB.tile([128, CS], F32, tag="psb")
                nc.tensor.matmul(db[0:4, :], obv_t[:, 0:4],
                                 embd[0][0:NBD, chs], start=True, stop=False)
                nc.tensor.matmul(db[0:4, :], obv_t[:, 4:8],
                                 embd[1][0:NBD, chs], start=False, stop=True)
                nc.vector.tensor_add(out=den[:, chs], in0=den[:, chs],
                                     in1=db[0:4, :])
            nc.vector.reciprocal(rd, den)
            nc.vector.tensor_mul(out=p1, in0=e1r, in1=rd)
            nc.vector.tensor_mul(out=p0, in0=e0r, in1=rd)

            # ============ PV + combine ============
            attn = [lt.tile([128, S], F32, tag="xq_attn", name=f"attn{i}") for i in range(2)]
            for dt in range(2):
                io = dt * 128
                for c in range(NCHUNK):
                    cl, chs = c * CS, slice(c * CS, (c + 1) * CS)
                    pv = psA.tile([128, CS], F32, tag="mmacc")
                    for hh in range(2):
                        nc.tensor.matmul(
                            pv[hh * 64:(hh + 1) * 64, :],
                            vbT[dt][hh * 64:hh * 64 + 48, :],
                            embd[dt][hh * 64:hh * 64 + 48, chs],
                            start=True, stop=True,
                            tile_position=(hh * 64, hh * 64))
                    p1b = psB.tile([128, CS], F32, tag="psb")
                    nc.tensor.matmul(p1b, ind_t[0:4, io:io + 128], p1[:, chs],
                                     start=True, stop=True)
                    p0b = psB.tile([128, CS], F32, tag="psb")
                    nc.tensor.matmul(p0b, ind_t[0:4, io:io + 128], p0[:, chs],
                                     start=True, stop=True)
                    rdb = psB.tile([128, CS], F32, tag="psb")
                    nc.tensor.matmul(rdb, ind_t[0:4, io:io + 128], rd[:, chs],
                                     start=True, stop=True)
                    m1 = mpool.tile([128, CS], F32, tag="mt")
                    nc.vector.tensor_mul(out=m1, in0=v_t[dt][:, chs], in1=p1b)
                    m2 = mpool.tile([128, CS], F32, tag="mt")
                    if c == 0:
                        nc.vector.memset(m2[:, 0:1], 0.0)
                        nc.vector.tensor_mul(out=m2[:, 1:CS],
                                             in0=v_t[dt][:, 0:CS - 1],
                                             in1=p0b[:, 1:CS])
                    else:
                        nc.vector.tensor_mul(out=m2,
                                             in0=v_t[dt][:, cl - 1:cl + CS - 1],
                                             in1=p0b)
                    pvs = mpool.tile([128, CS], F32, tag="mt")
                    nc.scalar.activation(pvs, pv, AF.Copy)
                    m3 = mpool.tile([128, CS], F32, tag="mt")
                    nc.vector.tensor_mul(out=m3, in0=pvs, in1=rdb)
                    m4 = mpool.tile([128, CS], F32, tag="mt")
                    nc.vector.tensor_add(out=m4, in0=m1, in1=m2)
                    nc.vector.tensor_add(out=attn[dt][:, chs], in0=m4, in1=m3)

            # ============ output projection ============
            attnr = [lt.tile([128, S], F32R, tag="xq_attn", name=f"attnr{i}")
                     for i in range(2)]
            for k in range(2):
                for c in range(NCHUNK):
                    chs = slice(c * CS, (c + 1) * CS)
                    if c % 2 == 0:
                        nc.scalar.activation(attnr[k][:, chs], attn[k][:, chs],
                                             AF.Copy)
                    else:
                        nc.vector.tensor_copy(out=attnr[k][:, chs],
                                              in_=attn[k][:, chs])
            for m in range(8):
                stage = kr[m % 2]  # kr tiles are dead by now; reuse as staging
                for c in range(NCHUNK):
                    chs = slice(c * CS, (c + 1) * CS)
                    acc = psA.tile([128, CS], F32, tag="mmacc")
                    for k in range(2):
                        nc.tensor.matmul(acc,
                                         wot_t[k][:, m * 128:(m + 1) * 128],
                                         attnr[k][:, chs],
                                         start=(k == 0), stop=(k == 1))
                    if c % 2 == 0:
                        nc.scalar.activation(stage[:, chs], acc, AF.Copy)
                    else:
                        nc.vector.tensor_copy(out=stage[:, chs], in_=acc)
                nc.sync.dma_start(out=outp[m * 128:(m + 1) * 128, :],
                                  in_=stage)
    _cap_sync_waits(nc)
    return nc


# ---------------- host side ----------------

def _host_consts(fc):
    C = np.zeros((128, S), np.float32)
    Sg = np.zeros((128, S), np.float32)
    for p in range(128):
        i = (p % 64) // 2
        if p % 2 == 0:
            C[p] = fc[:, i, 0, 0]
            Sg[p] = fc[:, i, 0, 1]
        else:
            C[p] = fc[:, i, 1, 1]
            Sg[p] = fc[:, i, 1, 0]
    starts = np.concatenate([[0], BND[:-1] + 1])
    nrs1 = np.ones(S, np.float32)
    nrs1[starts] = 0.0
    nrs = np.broadcast_to(nrs1, (128, S)).astype(np.float32).copy()
    mb = np.zeros((NBD, S), np.float32)
    t = np.arange(S)
    for hh in range(2):
        for jb in range(NBR):
            mb[hh * 64 + jb] = (t >= BND[jb] + 2).astype(np.float32)
    oblk = np.zeros((128, 8), np.float32)
    oblk[0:64, 0] = 1.0
    oblk[64:128, 1] = 1.0
    oblk[0:64, 4 + 2] = 1.0
    oblk[64:128, 4 + 3] = 1.0
    obv = np.zeros((NBD, 8), np.float32)
    obv[0:48, 0] = 1.0
    obv[64:112, 1] = 1.0
    obv[0:48, 4 + 2] = 1.0
    obv[64:112, 4 + 3] = 1.0
    ind4 = np.zeros((4, 256), np.float32)
    ind4[0, 0:64] = 1.0
    ind4[1, 64:128] = 1.0
    ind4[2, 128:192] = 1.0
    ind4[3, 192:256] = 1.0
    ind = np.zeros((128, 256), np.float32)
    for base in (0, 32, 64, 96):
        ind[base:base + 4] = ind4
    ident = np.eye(128, dtype=np.float32)
    return C, Sg, nrs, mb, oblk, obv, ind, ident


_prog = None


def make_in_maps(x, fc, wq_, wo_, a_k_, a_v_):
    C, Sg, nrs, mb, oblk, obv, ind, ident = _host_consts(fc)
    in_maps, metas = [], []
    for b in range(B):
        xT = np.ascontiguousarray(x[b].T)
        for g in range(NG):
            c0 = g * CH
            perm = np.concatenate([np.arange(c0, c0 + CH),
                                   np.arange(0, c0),
                                   np.arange(c0 + CH, D)]).astype(np.int64)
            xt_core = np.ascontiguousarray(xT[perm])
            wqt_core = np.ascontiguousarray(wq_[c0:c0 + CH, :].T[perm])
            wot_core = np.ascontiguousarray(wo_[:, c0:c0 + CH].T)
            apar = np.stack([a_k_[c0:c0 + 128], a_k_[c0 + 128:c0 + 256],
                             a_v_[c0:c0 + 128], a_v_[c0 + 128:c0 + 256]],
                            axis=1).astype(np.float32)
            in_maps.append({
                "xt": xt_core, "wqt": wqt_core, "wot": wot_core,
                "cbc": C, "sbc": Sg, "nrs": nrs, "apar": apar,
                "oblk": oblk, "obv": obv, "ind": ind, "maskb": mb,
                "ident": ident,
            })
            metas.append((b, g))
    return in_maps, metas


def kernel(x, freq_cis, wq, wo, a_k, a_v):
    global _prog
    x = np.asarray(x, np.float32)
    fc = np.asarray(freq_cis, np.float32)
    wq_ = np.asarray(wq, np.float32)
    wo_ = np.asarray(wo, np.float32)
    a_k_ = np.asarray(a_k, np.float32)
    a_v_ = np.asarray(a_v, np.float32)
    in_maps, metas = make_in_maps(x, fc, wq_, wo_, a_k_, a_v_)
    if _prog is None:
        _prog = build_program()
    res = run_bass_kernel_spmd(_prog, in_maps, core_ids=list(range(8)))
    out = np.zeros((B, S, D), np.float32)
    for (b, g), r in zip(metas, res.results):
        out[b] += r["outp"].T
    return out


if __name__ == "__main__":
    build_program()
    print("program built ok")

